# revision 5
# baseline (speedup 1.0000x reference)
"""AttnReadout kernel for Trainium2, 8 NeuronCores, data-parallel over batch.

Math (per batch b, head i):
  c[i,e]    = bu[i,e] + sum_d Wv[i,e,d] * x[b, i, last_nodes[b,i], d]
  z[t,e]    = sum_d x[b,t,d] * Wu[i,e,d]          (t ranges over O*N = 8192 tokens)
  s[t,e]    = sigmoid(z[t,e] + c[i,e])
  score[t]  = sum_e We[i,e] * s[t,e]
  alpha     = softmax(score)                       (shift-invariant; scores bounded ~|We|_1
                                                    so exp without max-subtraction is safe)
  out[b,i]  = sum_t alpha[t] * x[b,t,:]

Device dataflow per core (4 samples):
  - x in bf16, two DRAM layouts (host-prepared): transposed [d, t] for the
    projection, natural-chunked [t, d] for the weighted sum.
  - proj: PE matmul WuT[d,e] stationary x xT[d, 512] -> PSUM z[e, 512]
  - sigmoid: ACT with per-partition bias c -> SBUF s bf16 [e, 512]
  - score: PE matmul with s[e,128] as stationary (FWL), We[e,1] moving
    -> scores land token-on-partition in PSUM [128, 2, 64]
  - exp: ACT Exp with accum_out giving per-partition sums
  - Z: PE matmul zpart.T @ ones -> [2, 1]; DVE reciprocal
  - wsum: PE matmul alpha[t,2] stationary x xn[t,128] moving, PSUM-accumulated
  - out: ACT copy with per-partition scale Zinv -> DMA out
"""

import numpy as np
import ml_dtypes

import concourse.bacc as bacc
import concourse.tile as tile
from concourse import mybir
from concourse.bass_utils import run_bass_kernel_spmd

BF = ml_dtypes.bfloat16
B, O, N, D = 32, 2, 4096, 128
NCORES = 8
BPC = B // NCORES          # samples per core
T = O * N                  # tokens per sample
CH = 512                   # projection chunk (free dim)
NCH = T // CH              # 16
NC64 = T // 128            # 64 token chunks of 128


def _build_program():
    nc = bacc.Bacc("TRN2", target_bir_lowering=False)
    dt = mybir.dt
    f32, bf16 = dt.float32, dt.bfloat16

    xt_d = nc.dram_tensor("xt", [BPC, D, T], bf16, kind="ExternalInput")
    xn_d = nc.dram_tensor("xn", [BPC, 8, D, 8 * D], bf16, kind="ExternalInput")
    wu_d = nc.dram_tensor("wuT", [D, O, D], bf16, kind="ExternalInput")
    wv_d = nc.dram_tensor("wvT", [D, O, D], bf16, kind="ExternalInput")
    we_d = nc.dram_tensor("we2", [D, O], bf16, kind="ExternalInput")
    bu_d = nc.dram_tensor("bu2", [D, O], f32, kind="ExternalInput")
    xl_d = nc.dram_tensor("xlT", [D, O * BPC], bf16, kind="ExternalInput")
    on_d = nc.dram_tensor("ones", [D, 1], bf16, kind="ExternalInput")
    out_d = nc.dram_tensor("out", [BPC, O, D], f32, kind="ExternalOutput")

    Sig = mybir.ActivationFunctionType.Sigmoid
    Exp = mybir.ActivationFunctionType.Exp
    Ident = mybir.ActivationFunctionType.Identity
    Copy = mybir.ActivationFunctionType.Copy

    with tile.TileContext(nc) as tc:
        from contextlib import ExitStack

        with ExitStack() as ctx:
            singles = ctx.enter_context(tc.tile_pool(name="singles", bufs=1))
            xtp = ctx.enter_context(tc.tile_pool(name="xtp", bufs=2))
            xnp = ctx.enter_context(tc.tile_pool(name="xnp", bufs=2))
            zp = ctx.enter_context(tc.tile_pool(name="zp", bufs=3, space="PSUM"))
            sp = ctx.enter_context(tc.tile_pool(name="sp", bufs=4))
            scp = ctx.enter_context(tc.tile_pool(name="scp", bufs=2, space="PSUM"))
            up = ctx.enter_context(tc.tile_pool(name="up", bufs=2, space="PSUM"))
            mp = ctx.enter_context(tc.tile_pool(name="mp", bufs=1, space="PSUM"))
            smalls = ctx.enter_context(tc.tile_pool(name="smalls", bufs=2))

            wu_sb = singles.tile([D, O, D], bf16)
            nc.sync.dma_start(out=wu_sb, in_=wu_d[:])
            wv_sb = singles.tile([D, O, D], bf16)
            nc.sync.dma_start(out=wv_sb, in_=wv_d[:])
            we_sb = singles.tile([D, O], bf16)
            nc.sync.dma_start(out=we_sb, in_=we_d[:])
            bu_sb = singles.tile([D, O], f32)
            nc.sync.dma_start(out=bu_sb, in_=bu_d[:])
            xl_sb = singles.tile([D, O * BPC], bf16)
            nc.sync.dma_start(out=xl_sb, in_=xl_d[:])
            on_sb = singles.tile([D, 1], bf16)
            nc.sync.dma_start(out=on_sb, in_=on_d[:])

            # per-(sample, head) sigmoid bias c[e, j], j = i*BPC + b
            c_ps = mp.tile([D, O * BPC], f32, tag="misc")
            for i in range(O):
                nc.tensor.matmul(
                    c_ps[:, i * BPC : (i + 1) * BPC],
                    wv_sb[:, i, :],
                    xl_sb[:, i * BPC : (i + 1) * BPC],
                    start=True,
                    stop=True,
                )
            c_sb = singles.tile([D, O * BPC], f32)
            for i in range(O):
                nc.scalar.activation(
                    out=c_sb[:, i * BPC : (i + 1) * BPC],
                    in_=c_ps[:, i * BPC : (i + 1) * BPC],
                    func=Ident,
                    bias=bu_sb[:, i : i + 1],
                )

            for b in range(BPC):
                xt_sb = xtp.tile([D, T], bf16)
                for q in range(8):
                    nc.sync.dma_start(
                        out=xt_sb[:, q * (T // 8) : (q + 1) * (T // 8)],
                        in_=xt_d[b, :, q * (T // 8) : (q + 1) * (T // 8)],
                    )
                xn_sb = xnp.tile([D, NC64, D], bf16)
                for g in range(8):
                    nc.sync.dma_start(
                        out=xn_sb[:, g * 8 : (g + 1) * 8, :],
                        in_=xn_d[b, g].rearrange("p (c d) -> p c d", c=8),
                    )

                scores_ps = scp.tile([D, O, NC64], f32)
                for c in range(NCH):
                    for i in range(O):
                        z_ps = zp.tile([D, CH], f32)
                        nc.tensor.matmul(
                            z_ps,
                            wu_sb[:, i, :],
                            xt_sb[:, c * CH : (c + 1) * CH],
                            start=True,
                            stop=True,
                        )
                        s_sb = sp.tile([D, CH], bf16)
                        j = i * BPC + b
                        nc.scalar.activation(
                            out=s_sb, in_=z_ps, func=Sig, bias=c_sb[:, j : j + 1]
                        )
                        for sub in range(CH // D):
                            col = c * (CH // D) + sub
                            nc.tensor.matmul(
                                scores_ps[:, i, col : col + 1],
                                s_sb[:, sub * D : (sub + 1) * D],
                                we_sb[:, i : i + 1],
                                start=True,
                                stop=True,
                            )

                alpha_sb = smalls.tile([D, O, NC64], bf16, tag="alpha")
                zpart_sb = smalls.tile([D, O], f32, tag="zpart")
                for i in range(O):
                    nc.scalar.activation(
                        out=alpha_sb[:, i, :],
                        in_=scores_ps[:, i, :],
                        func=Exp,
                        accum_out=zpart_sb[:, i : i + 1],
                    )
                zpb_sb = smalls.tile([D, O], bf16, tag="zpb")
                nc.vector.tensor_copy(out=zpb_sb, in_=zpart_sb)
                z_ps2 = mp.tile([O, 1], f32, tag="misc")
                nc.tensor.matmul(z_ps2, zpb_sb, on_sb, start=True, stop=True)
                zinv_sb = smalls.tile([O, 1], f32, tag="zinv")
                nc.vector.reciprocal(out=zinv_sb, in_=z_ps2)

                u_ps = up.tile([O, D], f32)
                for c in range(NC64):
                    nc.tensor.matmul(
                        u_ps,
                        alpha_sb[:, :, c : c + 1].rearrange("p a b -> p (a b)"),
                        xn_sb[:, c, :],
                        start=(c == 0),
                        stop=(c == NC64 - 1),
                    )
                o_sb = smalls.tile([O, D], f32, tag="osb")
                nc.scalar.activation(
                    out=o_sb, in_=u_ps, func=Copy, scale=zinv_sb
                )
                nc.sync.dma_start(out=out_d[b], in_=o_sb)

    nc.compile()
    return nc


def _prep_core_inputs(x, Wu, bu, Wv, We, last_nodes):
    """Host-side input marshalling: dtype cast + layout. Returns per-core maps."""
    x = np.ascontiguousarray(x, dtype=np.float32)
    ln = np.asarray(last_nodes).astype(np.int64)
    xb = x.reshape(B, T, D)
    xbf = xb.astype(BF)                                  # [B, T, D] bf16
    xt = np.ascontiguousarray(xbf.transpose(0, 2, 1))    # [B, D, T]
    # natural-chunked layout: xn[b, g, p, cc*D + d] = xb[b, (g*8 + cc)*128 + p, d]
    xn = np.ascontiguousarray(
        xbf.reshape(B, 8, 8, D, D).transpose(0, 1, 3, 2, 4).reshape(B, 8, D, 8 * D)
    )
    # x_last gather, transposed: xlT[core][d, j], j = i*BPC + b_local
    xl = xb[np.arange(B)[:, None], ln + np.arange(O)[None, :] * N]   # [B, O, D] f32
    wuT = np.ascontiguousarray(Wu.transpose(0, 2, 1).astype(BF)).transpose(1, 0, 2)
    wvT = np.ascontiguousarray(Wv.transpose(0, 2, 1).astype(BF)).transpose(1, 0, 2)
    # wuT layout for DRAM [d, i, e]: element [d, i, e] = Wu[i, e, d]
    wuT = np.ascontiguousarray(wuT)
    wvT = np.ascontiguousarray(wvT)
    we2 = np.ascontiguousarray(We.astype(BF).T)          # [e, i]
    bu2 = np.ascontiguousarray(bu.astype(np.float32).T)  # [e, i]
    ones = np.ones((D, 1), BF)

    maps = []
    for core in range(NCORES):
        sl = slice(core * BPC, (core + 1) * BPC)
        xlc = xl[sl]                                     # [BPC, O, D]
        xlT = np.ascontiguousarray(
            xlc.transpose(2, 1, 0).reshape(D, O * BPC).astype(BF)
        )                                                # [d, i*BPC+b]
        maps.append(
            {
                "xt": xt[sl],
                "xn": xn[sl],
                "wuT": wuT,
                "wvT": wvT,
                "we2": we2,
                "bu2": bu2,
                "xlT": xlT,
                "ones": ones,
            }
        )
    return maps


_CACHE = {}
TRACE = False


def kernel(**inputs):
    x = np.asarray(inputs["x"])
    Wu = np.asarray(inputs["Wu"], dtype=np.float32)
    bu = np.asarray(inputs["bu"], dtype=np.float32)
    Wv = np.asarray(inputs["Wv"], dtype=np.float32)
    We = np.asarray(inputs["We"], dtype=np.float32)
    last_nodes = np.asarray(inputs["last_nodes"])

    maps = _prep_core_inputs(x, Wu, bu, Wv, We, last_nodes)
    if "nc" not in _CACHE:
        _CACHE["nc"] = _build_program()
    nc = _CACHE["nc"]
    res = run_bass_kernel_spmd(nc, maps, list(range(NCORES)), trace=TRACE)
    _CACHE["last_res"] = res
    outs = [np.asarray(r["out"], dtype=np.float32) for r in res.results]
    return np.concatenate(outs, axis=0)  # [B, O, D]


if __name__ == "__main__":
    rng = np.random.default_rng(0)
    x = rng.standard_normal((B, O, N, D), dtype=np.float32)
    Wu = rng.standard_normal((O, D, D), dtype=np.float32) * 0.09
    bu = np.zeros((O, D), np.float32)
    Wv = rng.standard_normal((O, D, D), dtype=np.float32) * 0.09
    We = rng.standard_normal((O, D), dtype=np.float32) * 0.09
    ln = rng.integers(0, N, size=(B, O)).astype(np.int64)
    out = kernel(x=x, Wu=Wu, bu=bu, Wv=Wv, We=We, last_nodes=ln)
    print(out.shape, out.dtype)


# revision 6
# speedup vs baseline: 1.0016x; 1.0016x over previous
"""AttnReadout kernel for Trainium2, 8 NeuronCores, data-parallel over batch.

Math (per batch b, head i):
  c[i,e]    = bu[i,e] + sum_d Wv[i,e,d] * x[b, i, last_nodes[b,i], d]
  z[t,e]    = sum_d x[b,t,d] * Wu[i,e,d]          (t over O*N = 8192 tokens)
  s[t,e]    = sigmoid(z[t,e] + c[i,e])
  score[t]  = sum_e We[i,e] * s[t,e]
  alpha     = softmax(score)        (scores bounded by |We|_1, so exp without
                                     max-subtraction is safe; softmax is
                                     shift-invariant so results match)
  out[b,i]  = sum_t alpha[t] * x[b,t,:]

Trick: sigmoid(v) = (1 + tanh(v/2))/2 and the We-dot is linear, so
  score = sum_e (We_e/2)*tanh((z_e + c_e)/2) + sum_e We_e/2
Using tanh keeps every ACT function (tanh, exp, identity) in the single
`exp_and_others` table set -> no ACT table reloads. The /2 factors are
folded into the uploaded weights (exact in bf16), the +sum(We)/2 into the
exp bias.

Device dataflow per core (4 samples):
  - x in bf16, two host-prepared DRAM layouts: transposed [d, t] for the
    projection, natural-chunked [t, d] for the weighted sum.
  - proj: PE matmul (Wu/2)^T stationary x xT[d, 512] -> PSUM z[e, 512]
  - tanh: ACT over [128, 1024] with per-partition bias ch -> SBUF bf16
  - score: PE matmul with tanh tile [e,128] stationary, (We/2)[e,1] moving
    -> scores land token-on-partition in PSUM
  - exp: ACT Exp(score + cw) with accum_out giving per-partition Z sums
  - Z: PE matmul zpart.T @ ones -> [2, 1]; DVE reciprocal
  - wsum: PE matmul alpha[t,2] stationary x xn[t,128] moving, accumulated
    into the same PSUM bank as the (already consumed) scores
  - out: DVE per-partition scale by 1/Z -> DMA out
"""

import numpy as np
import ml_dtypes

import concourse.bacc as bacc
import concourse.tile as tile
from concourse import mybir
from concourse.bass_utils import run_bass_kernel_spmd

BF = ml_dtypes.bfloat16
B, O, N, D = 32, 2, 4096, 128
NCORES = 8
BPC = B // NCORES          # samples per core
T = O * N                  # tokens per sample
CH = 512                   # projection chunk (free dim)
NG = T // (2 * CH)         # 8 tanh groups of 1024 per head
NC64 = T // 128            # 64 token chunks of 128


def _build_program():
    nc = bacc.Bacc("TRN2", target_bir_lowering=False)
    dt = mybir.dt
    f32, bf16 = dt.float32, dt.bfloat16

    xt_d = nc.dram_tensor("xt", [BPC, D, T], bf16, kind="ExternalInput")
    xn_d = nc.dram_tensor("xn", [BPC, 8, D, 8 * D], bf16, kind="ExternalInput")
    wu_d = nc.dram_tensor("wuT", [D, O, D], bf16, kind="ExternalInput")
    wv_d = nc.dram_tensor("wvT", [D, O, D], bf16, kind="ExternalInput")
    we_d = nc.dram_tensor("we2", [D, O], bf16, kind="ExternalInput")
    bu_d = nc.dram_tensor("bu2", [D, O], f32, kind="ExternalInput")
    cw_d = nc.dram_tensor("cw2", [D, O], f32, kind="ExternalInput")
    xl_d = nc.dram_tensor("xlT", [D, O * BPC], bf16, kind="ExternalInput")
    on_d = nc.dram_tensor("ones", [D, 1], bf16, kind="ExternalInput")
    out_d = nc.dram_tensor("out", [BPC, O, D], f32, kind="ExternalOutput")

    Tanh = mybir.ActivationFunctionType.Tanh
    Exp = mybir.ActivationFunctionType.Exp
    Ident = mybir.ActivationFunctionType.Identity

    with tile.TileContext(nc) as tc:
        from contextlib import ExitStack

        with ExitStack() as ctx:
            singles = ctx.enter_context(tc.tile_pool(name="singles", bufs=1))
            xtp = ctx.enter_context(tc.tile_pool(name="xtp", bufs=2))
            xnp = ctx.enter_context(tc.tile_pool(name="xnp", bufs=2))
            zp = ctx.enter_context(tc.tile_pool(name="zp", bufs=2, space="PSUM"))
            sp = ctx.enter_context(tc.tile_pool(name="sp", bufs=3))
            scp = ctx.enter_context(tc.tile_pool(name="scp", bufs=2, space="PSUM"))
            mp = ctx.enter_context(tc.tile_pool(name="mp", bufs=1, space="PSUM"))
            smalls = ctx.enter_context(tc.tile_pool(name="smalls", bufs=2))

            wu_sb = singles.tile([D, O, D], bf16)
            nc.sync.dma_start(out=wu_sb, in_=wu_d[:])
            wv_sb = singles.tile([D, O, D], bf16)
            nc.sync.dma_start(out=wv_sb, in_=wv_d[:])
            we_sb = singles.tile([D, O], bf16)
            nc.sync.dma_start(out=we_sb, in_=we_d[:])
            bu_sb = singles.tile([D, O], f32)
            nc.sync.dma_start(out=bu_sb, in_=bu_d[:])
            cw_sb = singles.tile([D, O], f32)
            nc.sync.dma_start(out=cw_sb, in_=cw_d[:])
            xl_sb = singles.tile([D, O * BPC], bf16)
            nc.sync.dma_start(out=xl_sb, in_=xl_d[:])
            on_sb = singles.tile([D, 1], bf16)
            nc.sync.dma_start(out=on_sb, in_=on_d[:])

            # per-(sample, head) tanh bias ch[e, j] = (xv + bu)/2, j = i*BPC + b
            # (wv and bu are uploaded pre-halved)
            c_ps = mp.tile([D, O * BPC], f32, tag="misc")
            for i in range(O):
                nc.tensor.matmul(
                    c_ps[:, i * BPC : (i + 1) * BPC],
                    wv_sb[:, i, :],
                    xl_sb[:, i * BPC : (i + 1) * BPC],
                    start=True,
                    stop=True,
                )
            ch_sb = singles.tile([D, O * BPC], f32)
            for i in range(O):
                nc.scalar.activation(
                    out=ch_sb[:, i * BPC : (i + 1) * BPC],
                    in_=c_ps[:, i * BPC : (i + 1) * BPC],
                    func=Ident,
                    bias=bu_sb[:, i : i + 1],
                )

            for b in range(BPC):
                xt_sb = xtp.tile([D, T], bf16)
                for q in range(8):
                    nc.sync.dma_start(
                        out=xt_sb[:, q * (T // 8) : (q + 1) * (T // 8)],
                        in_=xt_d[b, :, q * (T // 8) : (q + 1) * (T // 8)],
                    )
                xn_sb = xnp.tile([D, NC64, D], bf16)
                for g in range(8):
                    nc.sync.dma_start(
                        out=xn_sb[:, g * 8 : (g + 1) * 8, :],
                        in_=xn_d[b, g].rearrange("p (c d) -> p c d", c=8),
                    )

                # scores at [:, 0:128] (viewed [128, 2, 64]); u at [0:2, 128:256]
                scu = scp.tile([D, 2 * D], f32)
                scores = scu[:, 0:D].rearrange("p (i c) -> p i c", i=O)
                u_ap = scu[0:O, D : D + D]

                for i in range(O):
                    j = i * BPC + b
                    for g in range(NG):
                        z_ps = zp.tile([D, 2, CH], f32)
                        for h in range(2):
                            c = g * 2 + h
                            nc.tensor.matmul(
                                z_ps[:, h, :],
                                wu_sb[:, i, :],
                                xt_sb[:, c * CH : (c + 1) * CH],
                                start=True,
                                stop=True,
                            )
                        t_sb = sp.tile([D, 2, CH], bf16)
                        nc.scalar.activation(
                            out=t_sb, in_=z_ps, func=Tanh, bias=ch_sb[:, j : j + 1]
                        )
                        t_flat = t_sb.rearrange("p a b -> p (a b)")
                        for sub in range(2 * CH // D):
                            col = g * (2 * CH // D) + sub
                            nc.tensor.matmul(
                                scores[:, i, col : col + 1],
                                t_flat[:, sub * D : (sub + 1) * D],
                                we_sb[:, i : i + 1],
                                start=True,
                                stop=True,
                            )

                alpha_sb = smalls.tile([D, O, NC64], bf16, tag="alpha")
                zpart_sb = smalls.tile([D, O], f32, tag="zpart")
                for i in range(O):
                    nc.scalar.activation(
                        out=alpha_sb[:, i, :],
                        in_=scores[:, i, :],
                        func=Exp,
                        bias=cw_sb[:, i : i + 1],
                        accum_out=zpart_sb[:, i : i + 1],
                    )
                zpb_sb = smalls.tile([D, O], bf16, tag="zpb")
                nc.vector.tensor_copy(out=zpb_sb, in_=zpart_sb)
                z_ps2 = mp.tile([O, 1], f32, tag="misc")
                nc.tensor.matmul(z_ps2, zpb_sb, on_sb, start=True, stop=True)
                zinv_sb = smalls.tile([O, 1], f32, tag="zinv")
                nc.vector.reciprocal(out=zinv_sb, in_=z_ps2)

                for c in range(NC64):
                    nc.tensor.matmul(
                        u_ap,
                        alpha_sb[:, :, c : c + 1].rearrange("p a b -> p (a b)"),
                        xn_sb[:, c, :],
                        start=(c == 0),
                        stop=(c == NC64 - 1),
                    )
                o_sb = smalls.tile([O, D], f32, tag="osb")
                nc.vector.tensor_scalar_mul(o_sb, u_ap, zinv_sb)
                nc.sync.dma_start(out=out_d[b], in_=o_sb)

    nc.compile()
    return nc


def _prep_core_inputs(x, Wu, bu, Wv, We, last_nodes):
    """Host-side input marshalling: dtype cast + layout (weights pre-halved
    for the tanh formulation). Returns per-core input maps."""
    x = np.ascontiguousarray(x, dtype=np.float32)
    ln = np.asarray(last_nodes).astype(np.int64)
    xb = x.reshape(B, T, D)
    xbf = xb.astype(BF)                                  # [B, T, D] bf16
    xt = np.ascontiguousarray(xbf.transpose(0, 2, 1))    # [B, D, T]
    # natural-chunked layout: xn[b, g, p, cc*D + d] = xb[b, (g*8 + cc)*128 + p, d]
    xn = np.ascontiguousarray(
        xbf.reshape(B, 8, 8, D, D).transpose(0, 1, 3, 2, 4).reshape(B, 8, D, 8 * D)
    )
    # x_last gather, transposed: xlT[core][d, j], j = i*BPC + b_local
    xl = xb[np.arange(B)[:, None], ln + np.arange(O)[None, :] * N]   # [B, O, D] f32
    # wuT[d, i, e] = Wu[i, e, d] / 2  (tanh halving, exact in bf16)
    wuT = np.ascontiguousarray((Wu * 0.5).transpose(2, 0, 1).astype(BF))
    wvT = np.ascontiguousarray((Wv * 0.5).transpose(2, 0, 1).astype(BF))
    we2 = np.ascontiguousarray((We * 0.5).astype(BF).T)  # [e, i] = We[i, e]/2
    bu2 = np.ascontiguousarray((bu * 0.5).astype(np.float32).T)  # [e, i]
    # exp bias: cw[i] = sum_e We[i, e]/2, replicated on all partitions
    cw = np.float32(0.5) * We.astype(np.float32).sum(axis=1)     # [O]
    cw2 = np.ascontiguousarray(np.broadcast_to(cw[None, :], (D, O)).astype(np.float32))
    ones = np.ones((D, 1), BF)

    maps = []
    for core in range(NCORES):
        sl = slice(core * BPC, (core + 1) * BPC)
        xlc = xl[sl]                                     # [BPC, O, D]
        xlT = np.ascontiguousarray(
            xlc.transpose(2, 1, 0).reshape(D, O * BPC).astype(BF)
        )                                                # [d, i*BPC+b]
        maps.append(
            {
                "xt": xt[sl],
                "xn": xn[sl],
                "wuT": wuT,
                "wvT": wvT,
                "we2": we2,
                "bu2": bu2,
                "cw2": cw2,
                "xlT": xlT,
                "ones": ones,
            }
        )
    return maps


_CACHE = {}
TRACE = False


def kernel(**inputs):
    x = np.asarray(inputs["x"])
    Wu = np.asarray(inputs["Wu"], dtype=np.float32)
    bu = np.asarray(inputs["bu"], dtype=np.float32)
    Wv = np.asarray(inputs["Wv"], dtype=np.float32)
    We = np.asarray(inputs["We"], dtype=np.float32)
    last_nodes = np.asarray(inputs["last_nodes"])

    maps = _prep_core_inputs(x, Wu, bu, Wv, We, last_nodes)
    if "nc" not in _CACHE:
        _CACHE["nc"] = _build_program()
    nc = _CACHE["nc"]
    res = run_bass_kernel_spmd(nc, maps, list(range(NCORES)), trace=TRACE)
    _CACHE["last_res"] = res
    outs = [np.asarray(r["out"], dtype=np.float32) for r in res.results]
    return np.concatenate(outs, axis=0)  # [B, O, D]


if __name__ == "__main__":
    rng = np.random.default_rng(0)
    x = rng.standard_normal((B, O, N, D), dtype=np.float32)
    Wu = rng.standard_normal((O, D, D), dtype=np.float32) * 0.09
    bu = np.zeros((O, D), np.float32)
    Wv = rng.standard_normal((O, D, D), dtype=np.float32) * 0.09
    We = rng.standard_normal((O, D), dtype=np.float32) * 0.09
    ln = rng.integers(0, N, size=(B, O)).astype(np.int64)
    out = kernel(x=x, Wu=Wu, bu=bu, Wv=Wv, We=We, last_nodes=ln)
    print(out.shape, out.dtype)


# revision 18
# speedup vs baseline: 1.1792x; 1.1773x over previous
"""AttnReadout kernel for Trainium2, 8 NeuronCores, data-parallel over batch.

Math (per batch b, head i):
  c[i,e]    = bu[i,e] + sum_d Wv[i,e,d] * x[b, i, last_nodes[b,i], d]
  z[t,e]    = sum_d x[b,t,d] * Wu[i,e,d]          (t over O*N = 8192 tokens)
  s[t,e]    = sigmoid(z[t,e] + c[i,e])
  score[t]  = sum_e We[i,e] * s[t,e]
  alpha     = softmax(score)        (scores bounded by |We|_1, so exp without
                                     max-subtraction is safe; softmax is
                                     shift-invariant so results match)
  out[b,i]  = sum_t alpha[t] * x[b,t,:]

Trick: sigmoid(v) = (1 + tanh(v/2))/2 and the We-dot is linear, so
  score = sum_e (We_e/2)*tanh((z_e + c_e)/2) + sum_e We_e/2
Using tanh keeps every ACT function (tanh, exp, identity) in the single
`exp_and_others` table set -> no ACT table reloads. The /2 factors are
folded into the uploaded weights (exact in bf16), the +sum(We)/2 into the
exp bias.

Device dataflow per core (4 samples):
  - x in bf16, two host-prepared DRAM layouts: transposed [d, t] for the
    projection, natural-chunked [t, d] for the weighted sum.
  - proj: PE matmul (Wu/2)^T stationary x xT[d, 512] -> PSUM z[e, 512]
  - tanh: ACT over [128, 1024] with per-partition bias ch -> SBUF bf16
  - score: PE matmul with tanh tile [e,128] stationary, (We/2)[e,1] moving
    -> scores land token-on-partition in PSUM
  - exp: ACT Exp(score + cw) with accum_out giving per-partition Z sums
  - Z: PE matmul zpart.T @ ones -> [2, 1]; DVE reciprocal
  - wsum: PE matmul alpha[t,2] stationary x xn[t,128] moving, accumulated
    into the same PSUM bank as the (already consumed) scores
  - out: DVE per-partition scale by 1/Z -> DMA out
"""

import numpy as np
import ml_dtypes

import concourse.bacc as bacc
import concourse.tile as tile
from concourse import mybir
from concourse import bass_utils
from concourse.bass_utils import run_bass_kernel_spmd


def _enable_fwl():
    """walrus is invoked with --enable-ldw-opt=false, which leaves LDWEIGHTS
    at 1 element/cycle. This kernel is stationary-load bound (one 128-column
    bf16 stationary per 128 tokens for the score reduction), so fast weight
    load is a ~2x lever there. Rewrite the flag on the walrus command line."""
    return  # walrus rejects bass's standalone InstLdweights under ldw-opt
    if getattr(bass_utils, "_fwl_patched", False):
        return
    orig = bass_utils.run_command

    def patched(argv, **kwargs):
        argv = [
            "--enable-ldw-opt=true" if a == "--enable-ldw-opt=false" else a
            for a in argv
        ]
        return orig(argv, **kwargs)

    bass_utils.run_command = patched
    bass_utils._fwl_patched = True

BF = ml_dtypes.bfloat16
B, O, N, D = 32, 2, 4096, 128
NCORES = 8
BPC = B // NCORES          # samples per core
T = O * N                  # tokens per sample
CH = 512                   # projection chunk (free dim)
NG = T // (2 * CH)         # 8 tanh groups of 1024 per head
NC64 = T // 128            # 64 token chunks of 128


def _build_program():
    nc = bacc.Bacc("TRN2", target_bir_lowering=False)
    dt = mybir.dt
    f32, bf16 = dt.float32, dt.bfloat16

    xt_d = nc.dram_tensor("xt", [BPC, D, T], bf16, kind="ExternalInput")
    xn_d = nc.dram_tensor("xn", [BPC, 2, D, 32 * D], bf16, kind="ExternalInput")
    wu_d = nc.dram_tensor("wuT", [D, O, D], bf16, kind="ExternalInput")
    wv_d = nc.dram_tensor("wvT", [D, O, D], bf16, kind="ExternalInput")
    we_d = nc.dram_tensor("we2", [D, O], bf16, kind="ExternalInput")
    bu_d = nc.dram_tensor("bu2", [D, O], f32, kind="ExternalInput")
    cw_d = nc.dram_tensor("cw2", [D, O], f32, kind="ExternalInput")
    xl_d = nc.dram_tensor("xlT", [D, O * BPC], bf16, kind="ExternalInput")
    on_d = nc.dram_tensor("ones", [D, D], bf16, kind="ExternalInput")
    out_d = nc.dram_tensor("out", [BPC, D, O], f32, kind="ExternalOutput")

    Tanh = mybir.ActivationFunctionType.Tanh
    Exp = mybir.ActivationFunctionType.Exp
    Ident = mybir.ActivationFunctionType.Identity

    with tile.TileContext(nc) as tc:
        from contextlib import ExitStack

        with ExitStack() as ctx:
            singles = ctx.enter_context(tc.tile_pool(name="singles", bufs=1))
            xtp = ctx.enter_context(tc.tile_pool(name="xtp", bufs=2))
            xnp = ctx.enter_context(tc.tile_pool(name="xnp", bufs=2))
            zp = ctx.enter_context(tc.tile_pool(name="zp", bufs=2, space="PSUM"))
            sp = ctx.enter_context(tc.tile_pool(name="sp", bufs=3))
            scp = ctx.enter_context(tc.tile_pool(name="scp", bufs=2, space="PSUM"))
            mp = ctx.enter_context(tc.tile_pool(name="mp", bufs=1, space="PSUM"))
            smalls = ctx.enter_context(tc.tile_pool(name="smalls", bufs=2))

            wu_sb = singles.tile([D, O, D], bf16)
            nc.sync.dma_start(out=wu_sb, in_=wu_d[:])
            wv_sb = singles.tile([D, O, D], bf16)
            nc.sync.dma_start(out=wv_sb, in_=wv_d[:])
            we_sb = singles.tile([D, O], bf16)
            nc.sync.dma_start(out=we_sb, in_=we_d[:])
            bu_sb = singles.tile([D, O], f32)
            nc.sync.dma_start(out=bu_sb, in_=bu_d[:])
            cw_sb = singles.tile([D, O], f32)
            nc.sync.dma_start(out=cw_sb, in_=cw_d[:])
            xl_sb = singles.tile([D, O * BPC], bf16)
            nc.sync.dma_start(out=xl_sb, in_=xl_d[:])
            on_sb = singles.tile([D, D], bf16)
            nc.sync.dma_start(out=on_sb, in_=on_d[:])

            # per-(sample, head) tanh bias ch[e, j] = (xv + bu)/2, j = i*BPC + b
            # (wv and bu are uploaded pre-halved)
            c_ps = mp.tile([D, O * BPC], f32, tag="misc")
            for i in range(O):
                nc.tensor.matmul(
                    c_ps[:, i * BPC : (i + 1) * BPC],
                    wv_sb[:, i, :],
                    xl_sb[:, i * BPC : (i + 1) * BPC],
                    start=True,
                    stop=True,
                )
            ch_sb = singles.tile([D, O * BPC], f32)
            for i in range(O):
                nc.scalar.activation(
                    out=ch_sb[:, i * BPC : (i + 1) * BPC],
                    in_=c_ps[:, i * BPC : (i + 1) * BPC],
                    func=Ident,
                    bias=bu_sb[:, i : i + 1],
                )

            for b in range(BPC):
                xt_sb = xtp.tile([D, T], bf16)
                for q in range(4):
                    nc.sync.dma_start(
                        out=xt_sb[:, q * (T // 4) : (q + 1) * (T // 4)],
                        in_=xt_d[b, :, q * (T // 4) : (q + 1) * (T // 4)],
                    )
                xn_sb = xnp.tile([D, NC64, D], bf16)
                for g in range(2):
                    nc.sync.dma_start(
                        out=xn_sb[:, g * 32 : (g + 1) * 32, :],
                        in_=xn_d[b, g].rearrange("p (c d) -> p c d", c=32),
                    )

                # scores at [:, 0:128] (viewed [128, 2, 64]); u' [d, 2] after
                scu = scp.tile([D, D + O], f32)
                scores = scu[:, 0:D].rearrange("p (i c) -> p i c", i=O)
                u_ap = scu[:, D : D + O]

                for i in range(O):
                    j = i * BPC + b
                    for g in range(NG):
                        z_ps = zp.tile([D, 2, CH], f32)
                        for h in range(2):
                            c = g * 2 + h
                            nc.tensor.matmul(
                                z_ps[:, h, :],
                                wu_sb[:, i, :],
                                xt_sb[:, c * CH : (c + 1) * CH],
                                start=True,
                                stop=True,
                            )
                        t_sb = sp.tile([D, 2, CH], bf16)
                        nc.scalar.activation(
                            out=t_sb, in_=z_ps, func=Tanh, bias=ch_sb[:, j : j + 1]
                        )
                        t_flat = t_sb.rearrange("p a b -> p (a b)")
                        for sub in range(2 * CH // D):
                            col = g * (2 * CH // D) + sub
                            nc.tensor.matmul(
                                scores[:, i, col : col + 1],
                                t_flat[:, sub * D : (sub + 1) * D],
                                we_sb[:, i : i + 1],
                                start=True,
                                stop=True,
                            )

                alpha_sb = smalls.tile([D, O, NC64], bf16, tag="alpha")
                zpart_sb = smalls.tile([D, O], f32, tag="zpart")
                for i in range(O):
                    nc.scalar.activation(
                        out=alpha_sb[:, i, :],
                        in_=scores[:, i, :],
                        func=Exp,
                        bias=cw_sb[:, i : i + 1],
                        accum_out=zpart_sb[:, i : i + 1],
                    )
                zpb_sb = smalls.tile([D, O], bf16, tag="zpb")
                nc.vector.tensor_copy(out=zpb_sb, in_=zpart_sb)
                # Zb[m, i] = Z_i broadcast to every partition (all-ones stationary)
                zb_ps = mp.tile([D, O], f32, tag="misc")
                nc.tensor.matmul(zb_ps, on_sb, zpb_sb, start=True, stop=True)
                zinv_sb = smalls.tile([D, O], f32, tag="zinv")
                nc.vector.reciprocal(out=zinv_sb, in_=zb_ps)

                # u'[d, i] = sum_t x[t, d] * alpha[t, i], x chunks stationary
                for c in range(NC64):
                    nc.tensor.matmul(
                        u_ap,
                        xn_sb[:, c, :],
                        alpha_sb[:, :, c : c + 1].rearrange("p a b -> p (a b)"),
                        start=(c == 0),
                        stop=(c == NC64 - 1),
                    )
                o_sb = smalls.tile([D, O], f32, tag="osb")
                nc.vector.tensor_mul(o_sb, u_ap, zinv_sb)
                nc.sync.dma_start(out=out_d[b], in_=o_sb)

    nc.compile()
    return nc


def _prep_core_inputs(x, Wu, bu, Wv, We, last_nodes):
    """Host-side input marshalling: dtype cast + layout (weights pre-halved
    for the tanh formulation). Returns per-core input maps."""
    x = np.ascontiguousarray(x, dtype=np.float32)
    ln = np.asarray(last_nodes).astype(np.int64)
    xb = x.reshape(B, T, D)
    xbf = xb.astype(BF)                                  # [B, T, D] bf16
    xt = np.ascontiguousarray(xbf.transpose(0, 2, 1))    # [B, D, T]
    # natural-chunked layout: xn[b, g, p, cc*D + d] = xb[b, (g*32 + cc)*128 + p, d]
    xn = np.ascontiguousarray(
        xbf.reshape(B, 2, 32, D, D).transpose(0, 1, 3, 2, 4).reshape(B, 2, D, 32 * D)
    )
    # x_last gather, transposed: xlT[core][d, j], j = i*BPC + b_local
    xl = xb[np.arange(B)[:, None], ln + np.arange(O)[None, :] * N]   # [B, O, D] f32
    # wuT[d, i, e] = Wu[i, e, d] / 2  (tanh halving, exact in bf16)
    wuT = np.ascontiguousarray((Wu * 0.5).transpose(2, 0, 1).astype(BF))
    wvT = np.ascontiguousarray((Wv * 0.5).transpose(2, 0, 1).astype(BF))
    we2 = np.ascontiguousarray((We * 0.5).astype(BF).T)  # [e, i] = We[i, e]/2
    bu2 = np.ascontiguousarray((bu * 0.5).astype(np.float32).T)  # [e, i]
    # exp bias: cw[i] = sum_e We[i, e]/2, replicated on all partitions
    cw = np.float32(0.5) * We.astype(np.float32).sum(axis=1)     # [O]
    cw2 = np.ascontiguousarray(np.broadcast_to(cw[None, :], (D, O)).astype(np.float32))
    ones = np.ones((D, D), BF)

    maps = []
    for core in range(NCORES):
        sl = slice(core * BPC, (core + 1) * BPC)
        xlc = xl[sl]                                     # [BPC, O, D]
        xlT = np.ascontiguousarray(
            xlc.transpose(2, 1, 0).reshape(D, O * BPC).astype(BF)
        )                                                # [d, i*BPC+b]
        maps.append(
            {
                "xt": xt[sl],
                "xn": xn[sl],
                "wuT": wuT,
                "wvT": wvT,
                "we2": we2,
                "bu2": bu2,
                "cw2": cw2,
                "xlT": xlT,
                "ones": ones,
            }
        )
    return maps


_CACHE = {}
TRACE = False


def kernel(**inputs):
    x = np.asarray(inputs["x"])
    Wu = np.asarray(inputs["Wu"], dtype=np.float32)
    bu = np.asarray(inputs["bu"], dtype=np.float32)
    Wv = np.asarray(inputs["Wv"], dtype=np.float32)
    We = np.asarray(inputs["We"], dtype=np.float32)
    last_nodes = np.asarray(inputs["last_nodes"])

    _enable_fwl()
    maps = _prep_core_inputs(x, Wu, bu, Wv, We, last_nodes)
    if "nc" not in _CACHE:
        _CACHE["nc"] = _build_program()
    nc = _CACHE["nc"]
    res = run_bass_kernel_spmd(nc, maps, list(range(NCORES)), trace=TRACE)
    _CACHE["last_res"] = res
    outs = [
        np.asarray(r["out"], dtype=np.float32).transpose(0, 2, 1)
        for r in res.results
    ]
    return np.concatenate(outs, axis=0)  # [B, O, D]


if __name__ == "__main__":
    rng = np.random.default_rng(0)
    x = rng.standard_normal((B, O, N, D), dtype=np.float32)
    Wu = rng.standard_normal((O, D, D), dtype=np.float32) * 0.09
    bu = np.zeros((O, D), np.float32)
    Wv = rng.standard_normal((O, D, D), dtype=np.float32) * 0.09
    We = rng.standard_normal((O, D), dtype=np.float32) * 0.09
    ln = rng.integers(0, N, size=(B, O)).astype(np.int64)
    out = kernel(x=x, Wu=Wu, bu=bu, Wv=Wv, We=We, last_nodes=ln)
    print(out.shape, out.dtype)
